# revision 3
# baseline (speedup 1.0000x reference)
"""Correlation-layer kernel for Trainium2 (8 NeuronCores, data-parallel over batch).

Problem (per batch b):
    corr[k, m] = sum_c x[b, c, u, v] * y[b, c, i, j],  k = v*h+u, m = i*w+j
    out = relu(corr) / sqrt(sum_k relu(corr)^2 + eps)   (normalize over k per m)

Shapes: x, y = (8, 128, 48, 64) fp32 -> out (8, 3072, 48, 64) fp32.
Sharding: 1 batch per core. Per core it is a (3072x128)@(128x3072) matmul,
ReLU, and an L2 normalization over the 3072-channel dim.

Design ("N"): all tiles in natural layout (k on partitions, m on free dim).
  Phase A (per 512-wide m-chunk): 24 f32r matmuls -> psum; ACT relu -> fp16;
    DVE square (fp16, 2x mode); PE ones-matmul accumulates sum-of-squares over
    the 24 k-tiles into a [1, 512] psum row.
  Recip: transpose the ss row to a [128, 4] column tile (PE), sqrt (ACT) +
    reciprocal + one Newton rsqrt refinement (DVE), transpose back to a row,
    broadcast to [128, 512] via a rank-1 PE matmul.
  Phase B: scale the y-chunk by recip (DVE), re-matmul (f32r), and evacuate
    with plain ReLU split across ACT and DVE; DMA out per k-tile.
"""

import sys

sys.path.insert(0, "/opt/trn_rl_repo")

import numpy as np

_BUILD_CACHE = {}

B, C, H, W = 8, 128, 48, 64
K = W * H      # 3072 output channels, k = v*h+u
M = H * W      # 3072 spatial positions, m = i*w+j
CH = 512       # m-chunk width
NCH = M // CH  # 6 chunks
NKT = K // 128  # 24 k-tiles
EPS = 1e-6

# Phase-B evacuation split: ACT takes kt % 5 in ACT_KTS, DVE the rest.
ACT_KTS = (0, 2)


def build():
    from concourse import bacc, bass, mybir, tile

    F32 = mybir.dt.float32
    F32R = mybir.dt.float32r
    F16 = mybir.dt.float16
    AF = mybir.ActivationFunctionType
    OP = mybir.AluOpType

    nc = bacc.Bacc("TRN2", debug=False, target_bir_lowering=False)

    a_d = nc.dram_tensor("a", [C, K], F32R, kind="ExternalInput")
    b_d = nc.dram_tensor("b", [C, M], F32R, kind="ExternalInput")
    id_d = nc.dram_tensor("ident", [128, 128], F32, kind="ExternalInput")
    onc_d = nc.dram_tensor("onescol", [128, 1], F16, kind="ExternalInput")
    onr_d = nc.dram_tensor("onesrow", [1, 128], F32R, kind="ExternalInput")
    out_d = nc.dram_tensor("out", [K, M], F32, kind="ExternalOutput")

    with tile.TileContext(nc) as tc:
        with (
            tc.tile_pool(name="pers", bufs=1) as pers,
            tc.tile_pool(name="work", bufs=3) as work,
            tc.tile_pool(name="outp", bufs=8) as outp,
            tc.tile_pool(name="chain", bufs=2) as chain,
            tc.tile_pool(name="psA", bufs=2, space=bass.MemorySpace.PSUM) as psA,
            tc.tile_pool(name="psB", bufs=2, space=bass.MemorySpace.PSUM) as psB,
            tc.tile_pool(name="psU", bufs=3, space=bass.MemorySpace.PSUM) as psU,
        ):
            a_t = pers.tile([C, K], F32R)
            b_t = pers.tile([C, M], F32R)
            id_t = pers.tile([128, 128], F32)
            onc_t = pers.tile([128, 1], F16)
            onr_t = pers.tile([1, 128], F32R)
            nc.sync.dma_start(a_t[:], a_d[:])
            nc.sync.dma_start(b_t[:], b_d[:])
            nc.sync.dma_start(id_t[:], id_d[:])
            nc.sync.dma_start(onc_t[:], onc_d[:])
            nc.sync.dma_start(onr_t[:], onr_d[:])

            ss_rows = {}

            def emit_A(c):
                """Matmul chunk c, relu->fp16, square, ones-matmul reduce."""
                m0 = c * CH
                ss_ps = psU.tile([1, CH], F32, tag="u")
                sqs = []
                for kt in range(NKT):
                    pA = psA.tile([128, CH], F32, tag="pA")
                    nc.tensor.matmul(
                        pA[:], a_t[:, kt * 128 : (kt + 1) * 128],
                        b_t[:, m0 : m0 + CH], start=True, stop=True,
                    )
                    r16 = work.tile([128, CH], F16, tag="r16")
                    nc.scalar.activation(r16[:], pA[:], AF.Relu)
                    s16 = work.tile([128, CH], F16, tag="s16")
                    nc.vector.tensor_tensor(s16[:], r16[:], r16[:], OP.mult)
                    sqs.append(s16)
                for kt in range(NKT):
                    nc.tensor.matmul(
                        ss_ps[:], onc_t[:], sqs[kt][:],
                        start=(kt == 0), stop=(kt == NKT - 1),
                        skip_group_check=True,
                    )
                ss_rows[c] = ss_ps

            def emit_recip_and_B(c):
                m0 = c * CH
                ss_ps = ss_rows.pop(c)
                ss_row = chain.tile([1, CH], F32, tag="ssrow")
                nc.scalar.activation(ss_row[:], ss_ps[:], AF.Copy)
                # transpose the row into a [128, 4] column tile
                tpa_ps = psU.tile([128, 4], F32, tag="u")
                for j in range(4):
                    nc.tensor.transpose(
                        tpa_ps[:, j : j + 1],
                        ss_row[:, j * 128 : (j + 1) * 128], id_t[0:1, 0:1],
                    )
                ss_col = chain.tile([128, 4], F32, tag="sscol")
                nc.scalar.activation(ss_col[:], tpa_ps[:], AF.Copy)
                # recip = 1/sqrt(ss + eps), with one Newton rsqrt refinement
                xx = chain.tile([128, 4], F32, tag="xx")
                nc.vector.tensor_scalar_add(xx[:], ss_col[:], EPS)
                s0 = chain.tile([128, 4], F32, tag="s0")
                nc.scalar.activation(s0[:], xx[:], AF.Sqrt)
                y0 = chain.tile([128, 4], F32, tag="y0")
                nc.vector.reciprocal(y0[:], s0[:])
                t0 = chain.tile([128, 4], F32, tag="t0")
                nc.vector.tensor_tensor(t0[:], y0[:], y0[:], OP.mult)
                nc.vector.tensor_tensor(t0[:], t0[:], xx[:], OP.mult)
                nc.vector.tensor_scalar(
                    out=t0[:], in0=t0[:], scalar1=-0.5, scalar2=1.5,
                    op0=OP.mult, op1=OP.add,
                )
                rc = chain.tile([128, 4], F32, tag="rc")
                nc.vector.tensor_tensor(rc[:], y0[:], t0[:], OP.mult)
                # transpose back to a [1, 512] row
                tpb_ps = psU.tile([1, CH], F32, tag="u")
                for j in range(4):
                    nc.tensor.transpose(
                        tpb_ps[:, j * 128 : (j + 1) * 128], rc[:, j : j + 1],
                        id_t[:],
                    )
                r_row = chain.tile([1, CH], F32R, tag="rrow")
                nc.scalar.activation(r_row[:], tpb_ps[:], AF.Copy)
                # broadcast to all 128 partitions
                bc_ps = psU.tile([128, CH], F32, tag="u")
                nc.tensor.matmul(bc_ps[:], onr_t[:], r_row[:], start=True, stop=True)
                bc_sb = chain.tile([128, CH], F32, tag="bcsb")
                nc.scalar.activation(bc_sb[:], bc_ps[:], AF.Copy)
                # scale the y-chunk
                bs_t = chain.tile([128, CH], F32R, tag="bs")
                nc.vector.tensor_tensor(bs_t[:], b_t[:, m0 : m0 + CH], bc_sb[:], OP.mult)
                # Phase B: re-matmul with scaled y, relu-evacuate, DMA out
                for kt in range(NKT):
                    pB = psB.tile([128, CH], F32, tag="pB")
                    nc.tensor.matmul(
                        pB[:], a_t[:, kt * 128 : (kt + 1) * 128], bs_t[:],
                        start=True, stop=True,
                    )
                    o_t = outp.tile([128, CH], F32, tag="o")
                    if kt % 5 in ACT_KTS:
                        nc.scalar.activation(o_t[:], pB[:], AF.Relu)
                    else:
                        nc.vector.tensor_scalar(
                            out=o_t[:], in0=pB[:], scalar1=0.0, scalar2=None,
                            op0=OP.max, op1=OP.bypass,
                        )
                    nc.sync.dma_start(
                        out_d[kt * 128 : (kt + 1) * 128, m0 : m0 + CH], o_t[:]
                    )

            emit_A(0)
            for c in range(NCH):
                if c + 1 < NCH:
                    emit_A(c + 1)
                emit_recip_and_B(c)

    nc.compile()
    return nc


def get_built():
    if "nc" not in _BUILD_CACHE:
        _BUILD_CACHE["nc"] = build()
    return _BUILD_CACHE["nc"]


def make_in_maps(x, y):
    ident = np.eye(128, dtype=np.float32)
    onescol = np.ones((128, 1), dtype=np.float16)
    onesrow = np.ones((1, 128), dtype=np.float32)
    in_maps = []
    for bi in range(B):
        a = np.ascontiguousarray(
            np.asarray(x)[bi].transpose(0, 2, 1).reshape(C, K)
        ).astype(np.float32)
        bm = np.ascontiguousarray(np.asarray(y)[bi].reshape(C, M)).astype(np.float32)
        in_maps.append(
            {"a": a, "b": bm, "ident": ident, "onescol": onescol, "onesrow": onesrow}
        )
    return in_maps


def run(x, y, trace=False):
    from concourse import bass_utils

    nc = get_built()
    in_maps = make_in_maps(x, y)
    res = bass_utils.run_bass_kernel_spmd(
        nc, in_maps, core_ids=list(range(B)), trace=trace
    )
    out = np.stack([res.results[bi]["out"].reshape(K, H, W) for bi in range(B)])
    return out, res


def kernel(x, y):
    out, _ = run(x, y, trace=False)
    return out


# revision 4
# speedup vs baseline: 1.1358x; 1.1358x over previous
"""Correlation-layer kernel for Trainium2 (8 NeuronCores, data-parallel over batch).

Problem (per batch b):
    corr[k, m] = sum_c x[b, c, u, v] * y[b, c, i, j],  k = v*h+u, m = i*w+j
    out = relu(corr) / sqrt(sum_k relu(corr)^2 + eps)   (normalize over k per m)

Shapes: x, y = (8, 128, 48, 64) fp32 -> out (8, 3072, 48, 64) fp32.
Sharding: 1 batch per core. Per core it is a (3072x128)@(128x3072) matmul,
ReLU, and an L2 normalization over the 3072-channel dim.

Design (v2 "R"): natural layout (k on partitions, m on free dim), one matmul
pass. Per 512-wide m-chunk:
  - 24 f32r matmuls -> psum; ACT relu evacuates to fp16 tiles (kept in SBUF);
    DVE squares them (fp16, 2x mode); a PE ones-matmul accumulates the sum of
    squares over the 24 k-tiles into a [1, 512] psum row.
  - recip chain: transpose the ss row into a [128, 4] column (PE), sqrt (ACT)
    + reciprocal + Newton rsqrt refinement (DVE), transpose back, broadcast to
    [128, 512] fp16 via a rank-1 PE matmul.
  - output: DVE multiplies each kept relu tile by the broadcast recip (fp16,
    2x mode) into one [128, 24*512] fp16 tile; a single SWDGE DMA casts
    fp16 -> fp32 while scattering to the output layout.
"""

import sys

sys.path.insert(0, "/opt/trn_rl_repo")

import numpy as np

_BUILD_CACHE = {}

B, C, H, W = 8, 128, 48, 64
K = W * H      # 3072 output channels, k = v*h+u
M = H * W      # 3072 spatial positions, m = i*w+j
CH = 512       # m-chunk width
NCH = M // CH  # 6 chunks
NKT = K // 128  # 24 k-tiles
EPS = 1e-6


def build():
    from concourse import bacc, bass, mybir, tile

    F32 = mybir.dt.float32
    F32R = mybir.dt.float32r
    F16 = mybir.dt.float16
    AF = mybir.ActivationFunctionType
    OP = mybir.AluOpType

    nc = bacc.Bacc("TRN2", debug=False, target_bir_lowering=False)

    a_d = nc.dram_tensor("a", [C, K], F32R, kind="ExternalInput")
    b_d = nc.dram_tensor("b", [C, M], F32R, kind="ExternalInput")
    id_d = nc.dram_tensor("ident", [128, 128], F32, kind="ExternalInput")
    onc_d = nc.dram_tensor("onescol", [128, 1], F16, kind="ExternalInput")
    onr_d = nc.dram_tensor("onesrow", [1, 128], F32R, kind="ExternalInput")
    out_d = nc.dram_tensor("out", [K, M], F32, kind="ExternalOutput")

    with tile.TileContext(nc) as tc:
        with (
            tc.tile_pool(name="pers", bufs=1) as pers,
            tc.tile_pool(name="rkeep", bufs=52) as rkeep,
            tc.tile_pool(name="work", bufs=4) as work,
            tc.tile_pool(name="big", bufs=2) as bigp,
            tc.tile_pool(name="chain", bufs=2) as chain,
            tc.tile_pool(name="psA", bufs=4, space=bass.MemorySpace.PSUM) as psA,
            tc.tile_pool(name="psS", bufs=2, space=bass.MemorySpace.PSUM) as psS,
            tc.tile_pool(name="psU", bufs=2, space=bass.MemorySpace.PSUM) as psU,
        ):
            a_t = pers.tile([C, K], F32R)
            b_t = pers.tile([C, M], F32R)
            id_t = pers.tile([128, 128], F32)
            onc_t = pers.tile([128, 1], F16)
            onr_t = pers.tile([1, 128], F32R)
            nc.sync.dma_start(a_t[:], a_d[:])
            nc.sync.dma_start(b_t[:], b_d[:])
            nc.sync.dma_start(id_t[:], id_d[:])
            nc.sync.dma_start(onc_t[:], onc_d[:])
            nc.sync.dma_start(onr_t[:], onr_d[:])

            ss_rows = {}
            relus = {}

            def emit_A(c):
                """Matmuls for chunk c, relu->fp16 (kept), square, ss reduce."""
                m0 = c * CH
                ss_ps = psS.tile([1, CH], F32, tag="ss")
                sqs = []
                rl = []
                for kt in range(NKT):
                    pA = psA.tile([128, CH], F32, tag="pA")
                    nc.tensor.matmul(
                        pA[:], a_t[:, kt * 128 : (kt + 1) * 128],
                        b_t[:, m0 : m0 + CH], start=True, stop=True,
                    )
                    r16 = rkeep.tile([128, CH], F16, tag="r16")
                    nc.scalar.activation(r16[:], pA[:], AF.Relu)
                    rl.append(r16)
                    s16 = work.tile([128, CH], F16, tag="s16")
                    nc.vector.tensor_tensor(s16[:], r16[:], r16[:], OP.mult)
                    sqs.append(s16)
                for kt in range(NKT):
                    nc.tensor.matmul(
                        ss_ps[:], onc_t[:], sqs[kt][:],
                        start=(kt == 0), stop=(kt == NKT - 1),
                        skip_group_check=True,
                    )
                ss_rows[c] = ss_ps
                relus[c] = rl

            def emit_recip(c):
                """ss row -> fp16 broadcast reciprocal-norm tile [128, CH]."""
                ss_ps = ss_rows.pop(c)
                ss_row = chain.tile([1, CH], F32, tag="ssrow")
                nc.scalar.activation(ss_row[:], ss_ps[:], AF.Copy)
                tpa_ps = psU.tile([128, 4], F32, tag="u")
                for j in range(4):
                    nc.tensor.transpose(
                        tpa_ps[:, j : j + 1],
                        ss_row[:, j * 128 : (j + 1) * 128], id_t[0:1, 0:1],
                    )
                ss_col = chain.tile([128, 4], F32, tag="sscol")
                nc.scalar.activation(ss_col[:], tpa_ps[:], AF.Copy)
                xx = chain.tile([128, 4], F32, tag="xx")
                nc.vector.tensor_scalar_add(xx[:], ss_col[:], EPS)
                s0 = chain.tile([128, 4], F32, tag="s0")
                nc.scalar.activation(s0[:], xx[:], AF.Sqrt)
                y0 = chain.tile([128, 4], F32, tag="y0")
                nc.vector.reciprocal(y0[:], s0[:])
                t0 = chain.tile([128, 4], F32, tag="t0")
                nc.vector.tensor_tensor(t0[:], y0[:], y0[:], OP.mult)
                nc.vector.tensor_tensor(t0[:], t0[:], xx[:], OP.mult)
                nc.vector.tensor_scalar(
                    out=t0[:], in0=t0[:], scalar1=-0.5, scalar2=1.5,
                    op0=OP.mult, op1=OP.add,
                )
                rc = chain.tile([128, 4], F32, tag="rc")
                nc.vector.tensor_tensor(rc[:], y0[:], t0[:], OP.mult)
                tpb_ps = psU.tile([1, CH], F32, tag="u")
                for j in range(4):
                    nc.tensor.transpose(
                        tpb_ps[:, j * 128 : (j + 1) * 128], rc[:, j : j + 1],
                        id_t[:],
                    )
                r_row = chain.tile([1, CH], F32R, tag="rrow")
                nc.scalar.activation(r_row[:], tpb_ps[:], AF.Copy)
                bc_ps = psU.tile([128, CH], F32, tag="u")
                nc.tensor.matmul(bc_ps[:], onr_t[:], r_row[:], start=True, stop=True)
                bc16 = chain.tile([128, CH], F16, tag="bc16")
                nc.scalar.activation(bc16[:], bc_ps[:], AF.Copy)
                return bc16

            def emit_B(c, bc16):
                """Scale kept relu tiles by recip, single casting DMA out."""
                m0 = c * CH
                big16 = bigp.tile([128, NKT * CH], F16, tag="big")
                rl = relus.pop(c)
                for kt in range(NKT):
                    nc.vector.tensor_tensor(
                        big16[:, kt * CH : (kt + 1) * CH], rl[kt][:], bc16[:],
                        OP.mult,
                    )
                dst = out_d[:, m0 : m0 + CH].rearrange("(kt p) j -> p kt j", p=128)
                src = big16[:].rearrange("p (kt j) -> p kt j", j=CH)
                nc.gpsimd.dma_start(dst, src)

            emit_A(0)
            for c in range(NCH):
                bc16 = emit_recip(c)
                if c + 1 < NCH:
                    emit_A(c + 1)
                emit_B(c, bc16)

    nc.compile()
    return nc


def get_built():
    if "nc" not in _BUILD_CACHE:
        _BUILD_CACHE["nc"] = build()
    return _BUILD_CACHE["nc"]


def make_in_maps(x, y):
    ident = np.eye(128, dtype=np.float32)
    onescol = np.ones((128, 1), dtype=np.float16)
    onesrow = np.ones((1, 128), dtype=np.float32)
    in_maps = []
    for bi in range(B):
        a = np.ascontiguousarray(
            np.asarray(x)[bi].transpose(0, 2, 1).reshape(C, K)
        ).astype(np.float32)
        bm = np.ascontiguousarray(np.asarray(y)[bi].reshape(C, M)).astype(np.float32)
        in_maps.append(
            {"a": a, "b": bm, "ident": ident, "onescol": onescol, "onesrow": onesrow}
        )
    return in_maps


def run(x, y, trace=False):
    from concourse import bass_utils

    nc = get_built()
    in_maps = make_in_maps(x, y)
    res = bass_utils.run_bass_kernel_spmd(
        nc, in_maps, core_ids=list(range(B)), trace=trace
    )
    out = np.stack([res.results[bi]["out"].reshape(K, H, W) for bi in range(B)])
    return out, res


def kernel(x, y):
    out, _ = run(x, y, trace=False)
    return out
